# revision 1
# baseline (speedup 1.0000x reference)
"""Trainium2 Bass kernel for de-emphasis IIR: y[n] = x[n] + 0.97*y[n-1] along last axis.

Input: waveform (32, 2, 480000) f32 = 64 independent sequences of 480k samples.
Sharding: pure data parallel — 8 sequences per core across 8 NeuronCores.

Per core: the 8 sequences are split into 16 chunks each -> 128 partitions,
each owning a contiguous 30000-sample chunk. The recurrence y = c*y_prev + x
runs along the free dim with the hardware DVE scan (tensor_tensor_scan),
~2.125 ns/elem across 128 partitions. Chunk boundaries use an H-sample halo
warmup (0.97^720 ~ 3e-10, far below fp32 noise), so partitions are fully
independent and no cross-partition or cross-core communication is needed.

DMA structure (measured on HW): each HWDGE ring (SP=sync, ACT=scalar)
sustains ~205 GB/s; SDMA engines are latency-bound on pure reads
(~13 GB/s/engine) and only reach ~26 GB/s when read and write descriptors
interleave, capping mixed traffic at the ~370-395 GB/s HBM/NC limit.
So: loads ride SP, stores ride ACT, the first tiles are small so the
store stream starts ASAP (entering mixed mode early), and the last
stores split across both rings.
"""

import numpy as np

COEFF = 0.97

# Full-problem geometry (hardcoded; harness runs kernel() standalone).
N_CORES = 8
SEQ_TOTAL = 64  # 32*2
S = SEQ_TOTAL // N_CORES  # 8 sequences per core
N = 480000  # samples per sequence
K = 16  # chunks per sequence -> S*K = 128 partitions
H = 720  # halo (warmup) samples per chunk
# per-chunk tile widths; sum must be (N/K + H) = 30720. Small first tiles
# get the scan/store pipeline going early; small last tiles shrink the tail.
WIDTHS = (1280, 1280) + (2560,) * 10 + (1280, 1280)
BUFS = 8
NSS = 2
RAW = True  # use the raw-bacc builder (no TileContext overhead)
USE_SWDGE = False

_BUILD_CACHE = {}


def build_deemph(S, N, K, H, widths, coeff=COEFF, bufs=8, nss=2):
    """Build the Bass program for one core: x[S,N] -> y[S,N]."""
    import concourse.bacc as bacc
    import concourse.mybir as mybir
    from concourse.mybir import AluOpType
    from concourse.tile import TileContext

    C = N // K  # chunk length
    P = S * K  # partitions
    assert N % K == 0, (N, K)
    widths = list(widths)
    assert sum(widths) == C + H, (sum(widths), C, H)
    T = len(widths)
    Wmax = max(widths)
    assert widths[0] > H
    nss = min(nss, T - 1)
    f32 = mybir.dt.float32

    # tile i covers per-chunk positions [starts[i]-H, starts[i]-H+widths[i])
    starts = []
    p = 0
    for w in widths:
        starts.append(p - H)
        p += w

    nc = bacc.Bacc(trn_type="TRN2", debug=False)
    x = nc.dram_tensor("x", [S, N], f32, kind="ExternalInput")
    y = nc.dram_tensor("y", [S, N], f32, kind="ExternalOutput")
    # [K, S, C] views: DMA pairing maps (k, s) -> partition k*S + s
    xt = x[:].rearrange("s (k j) -> s k j", k=K).transpose((1, 0, 2))
    yt = y[:].rearrange("s (k j) -> s k j", k=K).transpose((1, 0, 2))

    with TileContext(nc) as tc:
        with (
            tc.tile_pool(name="cpool", bufs=1) as cpool,
            tc.tile_pool(name="xpool", bufs=bufs) as xpool,
            tc.tile_pool(name="ypool", bufs=bufs) as ypool,
        ):
            ctile = cpool.tile([P, 1], f32)
            nc.vector.memset(ctile[:, :], coeff)
            half = K // 2
            # all loads first: each engine's emission order is its ring's
            # FIFO order, so deferred store-halves must not precede loads.
            xtiles = []
            for i, w in enumerate(widths):
                xtile = xpool.tile([P, Wmax], f32, tag="xt")
                if i == 0:
                    # chunk 0 of each seq (partitions 0..S): zero warmup
                    nc.vector.memset(xtile[0:S, 0:H], 0.0)
                    nc.sync.dma_start(xtile[0:S, H:w], x[:, 0 : w - H])
                    nc.scalar.dma_start(
                        xtile[S:P, 0:H], xt[0 : K - 1, :, C - H : C]
                    )
                    nc.sync.dma_start(
                        xtile[S : half * S, H:w], xt[1:half, :, 0 : w - H]
                    )
                    nc.scalar.dma_start(
                        xtile[half * S : P, H:w], xt[half:K, :, 0 : w - H]
                    )
                else:
                    lo = starts[i]
                    nc.sync.dma_start(xtile[:, 0:w], xt[:, :, lo : lo + w])
                xtiles.append(xtile)
            ytiles = []
            prev_y = None
            for i, w in enumerate(widths):
                ytile = ypool.tile([P, Wmax], f32, tag="yt")
                init = 0.0 if i == 0 else prev_y
                nc.vector.tensor_tensor_scan(
                    ytile[:, 0:w],
                    ctile[:, 0:1].broadcast_to((P, w)),
                    xtiles[i][:, 0:w],
                    init,
                    AluOpType.mult,
                    AluOpType.add,
                )
                prev_y = ytile[:, w - 1 : w]
                ytiles.append(ytile)
            for i, w in enumerate(widths):
                lo = starts[i]
                if i == 0:
                    nc.scalar.dma_start(yt[:, :, 0 : w - H], ytiles[i][:, H:w])
                elif i < T - nss:
                    nc.scalar.dma_start(yt[:, :, lo : lo + w], ytiles[i][:, 0:w])
                else:
                    nc.scalar.dma_start(
                        yt[0:half, :, lo : lo + w], ytiles[i][0 : half * S, 0:w]
                    )
            # SP-ring halves of the last nss stores, after all SP loads
            for i in range(T - nss, T):
                w, lo = widths[i], starts[i]
                if i == 0:
                    continue
                nc.sync.dma_start(
                    yt[half:K, :, lo : lo + w], ytiles[i][half * S : P, 0:w]
                )
    nc.compile()
    return nc


def build_deemph_raw(S, N, K, H, widths, coeff=COEFF, bufs=8, nss=2, use_swdge=False):
    """Raw bacc builder: same pipeline as build_deemph but with hand-rolled
    semaphores instead of TileContext — saves Tile's entry barrier and
    ~12us exit drain/EVSEM butterfly.

    Engines: sync = load ring (+ final store halves), scalar = store ring
    (+ tile-0 load halves), vector = memsets + scans.
    Per-tile DMA semaphores (xsem/ysem, +16 per DMA, waits only at
    all-writers-done values) + a single scan_sem chain (+1 per scan).
    """
    import concourse.bacc as bacc
    import concourse.mybir as mybir
    from concourse.mybir import AluOpType

    C = N // K
    P = S * K
    assert N % K == 0
    widths = list(widths)
    assert sum(widths) == C + H
    T = len(widths)
    Wmax = max(widths)
    assert widths[0] > H
    nss = min(nss, T - 1)
    f32 = mybir.dt.float32

    starts = []
    p = 0
    for w in widths:
        starts.append(p - H)
        p += w

    assert nss <= bufs  # y-slot waits stay within ACT-only store range

    nc = bacc.Bacc(trn_type="TRN2", debug=False)
    x = nc.dram_tensor("x", [S, N], f32, kind="ExternalInput")
    y = nc.dram_tensor("y", [S, N], f32, kind="ExternalOutput")
    xt = x[:].rearrange("s (k j) -> s k j", k=K).transpose((1, 0, 2))
    yt = y[:].rearrange("s (k j) -> s k j", k=K).transpose((1, 0, 2))

    half = K // 2
    xbuf = nc.alloc_sbuf_tensor("xbuf", [P, bufs * Wmax], f32)
    ybuf = nc.alloc_sbuf_tensor("ybuf", [P, bufs * Wmax], f32)
    cbuf = nc.alloc_sbuf_tensor("cbuf", [P, 1], f32)

    def xsl(i):
        o = (i % bufs) * Wmax
        return xbuf[:, o : o + widths[i]]

    def ysl(i):
        o = (i % bufs) * Wmax
        return ybuf[:, o : o + widths[i]]

    # per-tile semaphores: every wait is at an "all writers done" value,
    # which is the only ordering the DMA completion model guarantees
    xsem = [nc.alloc_semaphore(f"xsem{i}") for i in range(T)]
    ysem = [nc.alloc_semaphore(f"ysem{i}") for i in range(T)]
    scan_sem = nc.alloc_semaphore("scan_sem")
    init_sem = nc.alloc_semaphore("init_sem")
    n_load = [2] + [1] * (T - 1)  # DMAs per x tile (tile 0: data + halo)
    n_store = [1 if i < T - nss else 2 for i in range(T)]

    with nc.Block() as block:

        nla = 0  # last-loads-on-ACT experiment: measured 113.5us vs 103.0us, keep off

        @block.sync
        def _(sync):
            for i, w in enumerate(widths):
                if i >= T - nla:
                    continue
                if i >= bufs:
                    sync.wait_ge(scan_sem, i - bufs + 1)
                xv = xsl(i)
                if i == 0:
                    # one 128-partition DMA covers the whole data region:
                    # xt[0, s, :] is x[s, :], so k=0 rows come along free
                    sync.dma_start(
                        xv[:, H:w], xt[:, :, 0 : w - H]
                    ).then_inc(xsem[0], 16)
                else:
                    lo = starts[i]
                    sync.dma_start(xv[:, 0:w], xt[:, :, lo : lo + w]).then_inc(
                        xsem[i], 16
                    )
            for i in range(T - nss, T):
                w, lo = widths[i], starts[i]
                sync.wait_ge(scan_sem, i + 1)
                sync.dma_start(
                    yt[half:K, :, lo : lo + w], ysl(i)[half * S : P, 0:w]
                ).then_inc(ysem[i], 16)
            for i in range(T):
                sync.wait_ge(ysem[i], 16 * n_store[i])

        @block.scalar
        def _(scalar):
            w = widths[0]
            xv = xsl(0)
            scalar.dma_start(
                xv[S:P, 0:H], xt[0 : K - 1, :, C - H : C]
            ).then_inc(xsem[0], 16)
            for i, w in enumerate(widths):
                lo = starts[i]
                if use_swdge and i % 2 == 1 and i < T - nss:
                    continue
                scalar.wait_ge(scan_sem, i + 1)
                if i == 0:
                    scalar.dma_start(
                        yt[:, :, 0 : w - H], ysl(0)[:, H:w]
                    ).then_inc(ysem[0], 16)
                elif i < T - nss:
                    scalar.dma_start(
                        yt[:, :, lo : lo + w], ysl(i)[:, 0:w]
                    ).then_inc(ysem[i], 16)
                else:
                    scalar.dma_start(
                        yt[0:half, :, lo : lo + w], ysl(i)[0 : half * S, 0:w]
                    ).then_inc(ysem[i], 16)
                # late loads ride the ACT ring's spare mid-stream capacity;
                # store i's scan_sem wait (>= i+1) already covers load
                # (i+bufs)'s slot-reuse requirement
                j = i + bufs
                if T - nla <= j < T:
                    lo2 = starts[j]
                    scalar.dma_start(
                        xsl(j)[:, 0 : widths[j]], xt[:, :, lo2 : lo2 + widths[j]]
                    ).then_inc(xsem[j], 16)
            for i in range(T):
                scalar.wait_ge(ysem[i], 16 * n_store[i])

        if use_swdge:

            @block.gpsimd
            def _(gpsimd):
                for i, w in enumerate(widths):
                    if not (i % 2 == 1 and i < T - nss):
                        continue
                    lo = starts[i]
                    gpsimd.wait_ge(scan_sem, i + 1)
                    gpsimd.dma_start(
                        yt[:, :, lo : lo + w], ysl(i)[:, 0:w]
                    ).then_inc(ysem[i], 16)
                for i in range(T):
                    gpsimd.wait_ge(ysem[i], 16 * n_store[i])

        @block.vector
        def _(vector):
            vector.memset(cbuf[:, :], coeff).then_inc(init_sem, 1)
            vector.memset(xsl(0)[0:S, 0:H], 0.0).then_inc(init_sem, 1)
            prev = None
            for i, w in enumerate(widths):
                if i == 0:
                    vector.wait_ge(init_sem, 2)
                else:
                    # scan i reads scan i-1's last column (initial); the DVE
                    # pipe needs the @complete sem, program order isn't enough
                    vector.wait_ge(scan_sem, i)
                vector.wait_ge(xsem[i], 16 * n_load[i])
                if i >= bufs:
                    vector.wait_ge(ysem[i - bufs], 16 * n_store[i - bufs])
                yv = ysl(i)
                vector.tensor_tensor_scan(
                    yv[:, 0:w],
                    cbuf[:, 0:1].broadcast_to((P, w)),
                    xsl(i)[:, 0:w],
                    0.0 if prev is None else prev,
                    AluOpType.mult,
                    AluOpType.add,
                ).then_inc(scan_sem, 1)
                prev = yv[:, w - 1 : w]

    nc.compile()
    return nc


def _get_nc():
    key = (S, N, K, H, WIDTHS, BUFS, NSS, RAW, USE_SWDGE)
    if key not in _BUILD_CACHE:
        if RAW:
            _BUILD_CACHE[key] = build_deemph_raw(S, N, K, H, WIDTHS, bufs=BUFS, nss=NSS, use_swdge=USE_SWDGE)
        else:
            _BUILD_CACHE[key] = build_deemph(S, N, K, H, WIDTHS, bufs=BUFS, nss=NSS)
    return _BUILD_CACHE[key]


def run(waveform: np.ndarray, **spmd_kwargs):
    """Run on 8 NeuronCores; returns (full_output, BassKernelResults)."""
    from concourse.bass_utils import run_bass_kernel_spmd

    waveform = np.asarray(waveform)
    orig_shape = waveform.shape
    x = np.ascontiguousarray(waveform.reshape(SEQ_TOTAL, N).astype(np.float32, copy=False))
    nc = _get_nc()
    in_maps = [{"x": x[S * c : S * (c + 1)]} for c in range(N_CORES)]
    res = run_bass_kernel_spmd(nc, in_maps, core_ids=list(range(N_CORES)), **spmd_kwargs)
    out = np.concatenate([r["y"] for r in res.results], axis=0)
    return out.reshape(orig_shape), res


def kernel(waveform: np.ndarray) -> np.ndarray:
    out, _ = run(waveform)
    return out



# revision 3
# speedup vs baseline: 1.3977x; 1.3977x over previous
"""Trainium2 Bass kernel for de-emphasis IIR: y[n] = x[n] + 0.97*y[n-1] along last axis.

Input: waveform (32, 2, 480000) f32 = 64 sequences of 480k samples.
Sharding: pure data parallel, 8 sequences per core across 8 NeuronCores.

Strategy (fp16 + radix-4 scan decomposition, all compute on device):
- I/O in fp16 (host converts; rel-err budget 2e-2 vs ~1e-3 incurred), which
  halves HBM traffic to ~7.7 MB each way per core (~38 us per DGE ring).
- Each core's 8 sequences are cut into 16 chunks of 30000 -> 128 partitions,
  one chunk each, independent via a 256-sample zero-init warmup halo
  (0.97^256 ~ 4e-4, far below the error budget).
- Radix-4: the host deinterleaves each chunk (+halo) into 4 phases
  x_j[m] = x[4m+j] (pure layout, no arithmetic). On device:
    prep (ACT mul c + DVE fp16 add per step):
        s1 = c*x0 + x1; s2 = c*s1 + x2; u = c*s2 + x3
    scan (DVE, the only engine with tensor_tensor_scan):
        ys[m] = c^4*ys[m-1] + u[m]        # = y[4m+3], quarter-length scan
    recovery (ACT mul + DVE fp16 add):
        y_j[m] = c^(j+1)*ys[m-1] + s_j[m]  # j=0,1,2 (s_0 = x0)
  DVE cost/sample: (6*0.539 + 2.10)/4 ~ 1.33 ns vs 2.10 direct; ACT ~ 1.30.
- Everything is SBUF-resident per chunk (fp16 arrays ~ 209 KB/partition),
  so there is no buffer recycling; tiles are just column ranges with two
  counting semaphores (one per compute engine) + per-tile load sems.
- Loads ride the ACT HWDGE queue (issued up front, no waits -> never blocks
  ACT compute); stores ride the SP queue (SP engine is idle and absorbs the
  scan/recovery waits). One DMA per tile covers all phases via a strided AP.
- ACT/DVE software pipeline, depth 4: iteration i runs
    ACT: load(i+2), m1(i), m2(i-1), m3(i-2), r0..r2(i-3)
    DVE: scan(i-3), s2(i-1), u(i-2), y0..y2(i-4), s1(i)
  so every cross-engine dependency is satisfied ~an iteration ahead.
"""

import numpy as np

COEFF = 0.97
N_CORES = 8
S = 8            # sequences per core
N = 480000       # samples per sequence
K = 16           # chunks per sequence -> S*K = 128 partitions
CHUNK = N // K   # 30000
R = 4            # radix
PH = CHUNK // R  # 7500 stored phase-cols per chunk
H = 64           # warmup halo in phase-cols (256 samples)
CHAIN = PH + H   # 7564
WIDTHS = (1280, 1792, 1792, 1792, 908)  # chain tile widths, sum = CHAIN
P = S * K        # 128 partitions

_BUILD_CACHE = {}


def build_deemph(widths=WIDTHS, coeff=COEFF):
    import concourse.bacc as bacc
    import concourse.mybir as mybir
    from concourse.mybir import AluOpType

    f16 = mybir.dt.float16
    f32 = mybir.dt.float32
    widths = list(widths)
    T = len(widths)
    WM = max(widths)
    assert sum(widths) == CHAIN, (sum(widths), CHAIN)
    assert widths[0] > H

    # tile t covers chain cols [starts[t], starts[t]+widths[t])
    starts = []
    p = 0
    for w in widths:
        starts.append(p)
        p += w

    c1, c2, c3, c4 = coeff, coeff**2, coeff**3, coeff**4

    nc = bacc.Bacc(trn_type="TRN2", debug=False)
    x = nc.dram_tensor("x", [S, K, R, CHAIN], f16, kind="ExternalInput")
    y = nc.dram_tensor("y", [S, K, R, PH], f16, kind="ExternalOutput")
    xt = x[:].rearrange("s k j m -> (s k) j m")
    yt = y[:].rearrange("s k j m -> (s k) j m")

    xb = nc.alloc_sbuf_tensor("xb", [P, R * CHAIN], f16)
    s1b = nc.alloc_sbuf_tensor("s1b", [P, CHAIN], f16)
    s2b = nc.alloc_sbuf_tensor("s2b", [P, CHAIN], f16)
    ub = nc.alloc_sbuf_tensor("ub", [P, CHAIN], f16)
    ysb = nc.alloc_sbuf_tensor("ysb", [P, CHAIN], f16)
    yob = nc.alloc_sbuf_tensor("yob", [P, 3 * PH], f16)
    msc = nc.alloc_sbuf_tensor("msc", [P, 6 * WM], f16)
    rsc = nc.alloc_sbuf_tensor("rsc", [P, 6 * WM], f16)
    cb4 = nc.alloc_sbuf_tensor("cb4", [P, 1], f32)

    def xph(j, a, b):
        return xb[:, j * CHAIN + a : j * CHAIN + b]

    def mslot(op, t, w):
        o = (op * 2 + (t & 1)) * WM
        return msc[:, o : o + w]

    def rslot(op, t, w):
        o = (op * 2 + (t & 1)) * WM
        return rsc[:, o : o + w]

    xsem = [nc.alloc_semaphore(f"xsem{t}") for t in range(T)]
    asem = nc.alloc_semaphore("asem")   # +1 per ACT compute op
    dsem = nc.alloc_semaphore("dsem")   # +1 per DVE op
    ssem = nc.alloc_semaphore("ssem")   # +16 per store DMA

    # Build the iteration schedule (op kind, tile) and per-op global indices.
    act_sched, dve_sched, sp_sched = [], [], []
    for i in range(T + 4):
        if i + 2 < T:
            act_sched.append(("L", i + 2))
        if i < T:
            act_sched.append(("m1", i))
        if 0 <= i - 1 < T:
            act_sched.append(("m2", i - 1))
        if 0 <= i - 2 < T:
            act_sched.append(("m3", i - 2))
        if 0 <= i - 3 < T:
            act_sched.append(("r", i - 3))   # r0,r1,r2 emitted together
        if 0 <= i - 3 < T:
            dve_sched.append(("scan", i - 3))
        if 0 <= i - 1 < T:
            dve_sched.append(("s2", i - 1))
        if 0 <= i - 2 < T:
            dve_sched.append(("u", i - 2))
        if 0 <= i - 4 < T:
            dve_sched.append(("y", i - 4))   # y0,y1,y2 together
        if i < T:
            dve_sched.append(("s1", i))
        if 0 <= i - 3 < T:
            sp_sched.append(("Sy3", i - 3))
        if 0 <= i - 4 < T:
            sp_sched.append(("Sy012", i - 4))

    # Global completion index (1-based sem threshold) for each compute op.
    aidx, didx = {}, {}
    n = 0
    for kind, t in act_sched:
        if kind == "L":
            continue
        c = 3 if kind == "r" else 1
        n += c
        aidx[(kind, t)] = n  # value of asem after this op (group) completes
    n = 0
    for kind, t in dve_sched:
        c = 3 if kind == "y" else 1
        n += c
        didx[(kind, t)] = n

    n_stores = 2 * T

    with nc.Block() as block:

        @block.scalar
        def _(scalar):
            # first two loads up front; the rest interleave via the schedule
            for t in range(min(2, T)):
                lo, w = starts[t], widths[t]
                scalar.dma_start(
                    xb[:].rearrange("p (j m) -> p j m", j=R)[:, :, lo : lo + w],
                    xt[:, :, lo : lo + w],
                ).then_inc(xsem[t], 16)
            for kind, t in act_sched:
                lo, w = starts[t], widths[t]
                hi = lo + w
                sa = max(lo, H)          # first stored chain col in tile
                wr = hi - sa             # recovery width
                if kind == "L":
                    scalar.dma_start(
                        xb[:].rearrange("p (j m) -> p j m", j=R)[:, :, lo:hi],
                        xt[:, :, lo:hi],
                    ).then_inc(xsem[t], 16)
                elif kind == "m1":
                    scalar.wait_ge(xsem[t], 16)
                    scalar.mul(mslot(0, t, w), xph(0, lo, hi), c1).then_inc(asem, 1)
                elif kind == "m2":
                    scalar.wait_ge(dsem, didx[("s1", t)])
                    scalar.mul(mslot(1, t, w), s1b[:, lo:hi], c1).then_inc(asem, 1)
                elif kind == "m3":
                    scalar.wait_ge(dsem, didx[("s2", t)])
                    scalar.mul(mslot(2, t, w), s2b[:, lo:hi], c1).then_inc(asem, 1)
                elif kind == "r":
                    scalar.wait_ge(dsem, didx[("scan", t)])
                    ysv = ysb[:, sa - 1 : hi - 1]
                    scalar.mul(rslot(0, t, wr), ysv, c1).then_inc(asem, 1)
                    scalar.mul(rslot(1, t, wr), ysv, c2).then_inc(asem, 1)
                    scalar.mul(rslot(2, t, wr), ysv, c3).then_inc(asem, 1)

        @block.vector
        def _(vector):
            vector.memset(cb4[:, :], c4)
            for kind, t in dve_sched:
                lo, w = starts[t], widths[t]
                hi = lo + w
                sa = max(lo, H)
                wr = hi - sa
                ya, yb_ = sa - H, hi - H  # cols in the y output arrays
                if kind == "s1":
                    vector.wait_ge(asem, aidx[("m1", t)])
                    vector.tensor_tensor(
                        s1b[:, lo:hi], mslot(0, t, w), xph(1, lo, hi), AluOpType.add
                    ).then_inc(dsem, 1)
                elif kind == "s2":
                    vector.wait_ge(asem, aidx[("m2", t)])
                    vector.tensor_tensor(
                        s2b[:, lo:hi], mslot(1, t, w), xph(2, lo, hi), AluOpType.add
                    ).then_inc(dsem, 1)
                elif kind == "u":
                    vector.wait_ge(asem, aidx[("m3", t)])
                    vector.tensor_tensor(
                        ub[:, lo:hi], mslot(2, t, w), xph(3, lo, hi), AluOpType.add
                    ).then_inc(dsem, 1)
                elif kind == "scan":
                    init = 0.0 if t == 0 else ysb[:, lo - 1 : lo]
                    vector.tensor_tensor_scan(
                        ysb[:, lo:hi],
                        cb4[:, 0:1].broadcast_to((P, w)),
                        ub[:, lo:hi],
                        init,
                        AluOpType.mult,
                        AluOpType.add,
                    ).then_inc(dsem, 1)
                elif kind == "y":
                    vector.wait_ge(asem, aidx[("r", t)])
                    vector.tensor_tensor(
                        yob[:, ya : ya + wr], rslot(0, t, wr), xph(0, sa, hi),
                        AluOpType.add,
                    ).then_inc(dsem, 1)
                    vector.tensor_tensor(
                        yob[:, PH + ya : PH + ya + wr], rslot(1, t, wr),
                        s1b[:, sa:hi], AluOpType.add,
                    ).then_inc(dsem, 1)
                    vector.tensor_tensor(
                        yob[:, 2 * PH + ya : 2 * PH + ya + wr], rslot(2, t, wr),
                        s2b[:, sa:hi], AluOpType.add,
                    ).then_inc(dsem, 1)

        @block.sync
        def _(sync):
            for kind, t in sp_sched:
                lo, w = starts[t], widths[t]
                hi = lo + w
                sa = max(lo, H)
                wr = hi - sa
                ya = sa - H
                if kind == "Sy3":
                    sync.wait_ge(dsem, didx[("scan", t)])
                    sync.dma_start(
                        yt[:, 3:4, ya : ya + wr],
                        ysb[:, sa:hi].rearrange("p (j m) -> p j m", j=1),
                    ).then_inc(ssem, 16)
                else:  # Sy012
                    sync.wait_ge(dsem, didx[("y", t)])
                    sync.dma_start(
                        yt[:, 0:3, ya : ya + wr],
                        yob[:].rearrange("p (j m) -> p j m", j=3)[
                            :, :, ya : ya + wr
                        ],
                    ).then_inc(ssem, 16)
            sync.wait_ge(ssem, 16 * n_stores)

    nc.compile()
    return nc


def _get_nc():
    key = (WIDTHS,)
    if key not in _BUILD_CACHE:
        _BUILD_CACHE[key] = build_deemph(WIDTHS)
    return _BUILD_CACHE[key]


def _pack_core(xrows: np.ndarray) -> np.ndarray:
    """[S, N] f32 -> [S, K, R, CHAIN] f16 phase layout with warmup halo."""
    xpad = np.zeros((S, N + R * H), dtype=np.float16)
    xpad[:, R * H :] = xrows
    win = np.lib.stride_tricks.as_strided(
        xpad,
        shape=(S, K, CHUNK + R * H),
        strides=(xpad.strides[0], CHUNK * xpad.strides[1], xpad.strides[1]),
    )
    return np.ascontiguousarray(
        win.reshape(S, K, CHAIN, R).transpose(0, 1, 3, 2)
    )


def _unpack_core(y_all: np.ndarray) -> np.ndarray:
    """[S, K, R, PH] f16 -> [S, N] f32."""
    return (
        y_all.transpose(0, 1, 3, 2).reshape(S, N).astype(np.float32)
    )


def run(waveform: np.ndarray, **spmd_kwargs):
    """Run on 8 NeuronCores; returns (full_output, BassKernelResults)."""
    from concourse.bass_utils import run_bass_kernel_spmd

    waveform = np.asarray(waveform)
    orig_shape = waveform.shape
    xf = waveform.reshape(N_CORES * S, N)
    nc = _get_nc()
    in_maps = [
        {"x": _pack_core(xf[S * c : S * (c + 1)])} for c in range(N_CORES)
    ]
    res = run_bass_kernel_spmd(nc, in_maps, core_ids=list(range(N_CORES)), **spmd_kwargs)
    out = np.concatenate(
        [_unpack_core(r["y"]) for r in res.results], axis=0
    )
    return out.reshape(orig_shape), res


def kernel(waveform: np.ndarray) -> np.ndarray:
    out, _ = run(waveform)
    return out
